# revision 10
# baseline (speedup 1.0000x reference)
"""Trainium2 Bass kernel for nn_ConvModule (dense_cnn) — Winograd F(4,5).

Data-parallel over batch: 16 batch elems -> 8 cores x 2.
Per batch elem:
  dcnn conv (K=5)  -> Winograd F(4,5), fp32r transform-domain operands
  column LayerNorm (global LN over (C,S) cancels against per-column LN)
  w1 pointwise + Silu, computed transposed (s on partitions) in 124-row
    blocks that feed the conv2 Winograd input transform directly
  conv2 (K=5)      -> Winograd F(4,5), bf16 transform-domain operands
  GLU + BatchNorm (sync via AllReduce; folded into w3) + pointwise w3.

Winograd: y = A^T[(G g) . (B^T d)], m=4, r=5, points {0,±1,±2,±1/2,inf}.
Input transform runs on the tensor engine as matmuls against a constant
banded matrix (B packed per 128-row block of the padded u=s+2 axis);
output transform A^T is factored into pair sums/differences on DVE.
"""
import sys
import numpy as np
from contextlib import ExitStack

sys.path.insert(0, "/opt/trn_rl_repo")

import concourse.bass as bass
import concourse.tile as tile
from concourse import bacc, mybir
from concourse.bass_utils import run_bass_kernel_spmd

F32 = mybir.dt.float32
F32R = mybir.dt.float32r
BF16 = mybir.dt.bfloat16
AF = mybir.ActivationFunctionType
OP = mybir.AluOpType

B, S, C, K = 16, 2048, 512, 5
NCORES = 8
BL = B // NCORES          # 2 batch elems per core
CT = C // 128             # 4 c-tiles
C2T = 2 * C // 128        # 8 2c-tiles
NT = S // 4               # 512 winograd tiles per batch row
NB = 17                   # 124-stride blocks covering u in [0, 2112)
TPB = 31                  # full tiles per block
NTF = 8 * TPB             # transform psum free size (t, nn)
XW = 288                  # X~ half-tile n width (per-half local cols)
YW = 544                  # ypad plane width (>= 31*16+32 = 528)
EPS = 1e-5

# Winograd F(4,5), points [0, 1, -1, 2, -2, 1/2, -1/2] + inf.
BT_W = np.array([
    [-1, 0, 5.25, 0, -5.25, 0, 1, 0],
    [0, 1, 1, -4.25, -4.25, 1, 1, 0],
    [0, -1, 1, 4.25, -4.25, -1, 1, 0],
    [0, 0.5, 0.25, -2.5, -1.25, 2, 1, 0],
    [0, -0.5, 0.25, 2.5, -1.25, -2, 1, 0],
    [0, 2, 4, -2.5, -5, 0.5, 1, 0],
    [0, -2, 4, 2.5, -5, -0.5, 1, 0],
    [0, -1, 0, 5.25, 0, -5.25, 0, 1]], np.float64)
G_W = np.array([
    [-1, 0, 0, 0, 0],
    [-2/9, -2/9, -2/9, -2/9, -2/9],
    [-2/9, 2/9, -2/9, 2/9, -2/9],
    [1/90, 1/45, 2/45, 4/45, 8/45],
    [1/90, -1/45, 2/45, -4/45, 8/45],
    [32/45, 16/45, 8/45, 4/45, 2/45],
    [32/45, -16/45, 8/45, -4/45, 2/45],
    [0, 0, 0, 0, 1]], np.float64)
# A^T rows (j=0..3) over taps t=0..7:
#  j0 = Y0 + P1 + P2 + P3
#  j1 = M1 + 2*M2 + 0.5*M3
#  j2 = P1 + 4*P2 + 0.25*P3
#  j3 = M1 + 8*M2 + 0.125*M3 + Y7
# with Pk = Y(2k-1)+Y(2k), Mk = Y(2k-1)-Y(2k).

LAST_RESULT = None
_NC = None


def _build():
    nc = bacc.Bacc("TRN2", target_bir_lowering=False, debug=False,
                   num_devices=NCORES)

    xt_d = nc.dram_tensor("xt", [BL, NB, 128, C], BF16,
                          kind="ExternalInput").ap()
    waf_d = nc.dram_tensor("waf", [CT, 8, 128, CT, 128], F32R,
                           kind="ExternalInput").ap()
    w1t_d = nc.dram_tensor("w1t", [128, CT, 2 * C], BF16,
                           kind="ExternalInput").ap()
    w2f_d = nc.dram_tensor("w2f", [C2T, 8, 128, C2T, 128], BF16,
                           kind="ExternalInput").ap()
    w3_d = nc.dram_tensor("w3t", [128, CT, 512], BF16,
                          kind="ExternalInput").ap()
    bx_d = nc.dram_tensor("bx", [128, NTF], BF16, kind="ExternalInput").ap()
    bh_d = nc.dram_tensor("bh", [128, NTF], BF16, kind="ExternalInput").ap()
    b1bc_d = nc.dram_tensor("b1bc", [128, 2 * C], F32,
                            kind="ExternalInput").ap()
    dcnnb_d = nc.dram_tensor("dcnnb", [128, CT], F32, kind="ExternalInput").ap()
    b2_d = nc.dram_tensor("b2p", [128, C2T], F32, kind="ExternalInput").ap()
    b3_d = nc.dram_tensor("b3p", [1, 512], F32, kind="ExternalInput").ap()
    out_d = nc.dram_tensor("out", [BL, 4, NT, C], F32,
                           kind="ExternalOutput").ap()

    bn_in = [nc.dram_tensor(f"bn_in{j}", [128, 2], F32) for j in range(CT)]
    bn_out = [nc.dram_tensor(f"bn_out{j}", [128, 2], F32) for j in range(CT)]

    with tile.TileContext(nc) as tc, ExitStack() as ctx:
        cpool = ctx.enter_context(tc.tile_pool(name="const", bufs=1))
        pp_main = ctx.enter_context(tc.tile_pool(name="ppm", bufs=2, space="PSUM"))
        pp_tf = ctx.enter_context(tc.tile_pool(name="ppt", bufs=2, space="PSUM"))
        pp_stat = ctx.enter_context(tc.tile_pool(name="pps", bufs=1, space="PSUM"))
        pp_misc = ctx.enter_context(tc.tile_pool(name="ppx", bufs=2, space="PSUM"))

        # ---- constants ----
        bx_s = cpool.tile([128, NTF], BF16)
        nc.sync.dma_start(out=bx_s, in_=bx_d)
        bh_s = cpool.tile([128, NTF], BF16)
        nc.sync.dma_start(out=bh_s, in_=bh_d)
        w1t_s = cpool.tile([128, CT, 2 * C], BF16)
        nc.sync.dma_start(out=w1t_s, in_=w1t_d)
        w3t_s = cpool.tile([128, CT, 512], BF16)
        nc.sync.dma_start(out=w3t_s, in_=w3_d)
        b1bc_s = cpool.tile([128, 2 * C], F32)
        nc.sync.dma_start(out=b1bc_s, in_=b1bc_d)
        dcnnb_s = cpool.tile([128, CT], F32)
        nc.sync.dma_start(out=dcnnb_s, in_=dcnnb_d)
        b2_s = cpool.tile([128, C2T], F32)
        nc.sync.dma_start(out=b2_s, in_=b2_d)
        b3_s = cpool.tile([1, 512], F32)
        nc.sync.dma_start(out=b3_s, in_=b3_d)
        ones1_f = cpool.tile([1, 128], F32)
        nc.vector.memset(ones1_f, 1.0)
        ones1_s = cpool.tile([1, 128], BF16)
        nc.scalar.activation(ones1_s, ones1_f, AF.Copy)
        onesc_f = cpool.tile([128, 1], F32)
        nc.vector.memset(onesc_f, 1.0)
        onesc_s = cpool.tile([128, 1], BF16)
        nc.scalar.activation(onesc_s, onesc_f, AF.Copy)
        eps1_s = cpool.tile([1, 1], F32)
        nc.vector.memset(eps1_s, EPS)
        epsb_s = cpool.tile([128, 1], F32)
        nc.vector.memset(epsb_s, EPS)
        st6 = [cpool.tile([128, BL * 8, 6], F32, tag=f"st6_{j}",
                          name=f"st6_{j}") for j in range(CT)]
        bnpack_s = cpool.tile([128, CT, 2], F32)
        bnsum_s = cpool.tile([128, CT, 2], F32)
        rs_s = cpool.tile([128, CT], F32)
        mu_s = cpool.tile([128, CT], F32)
        murs_s = cpool.tile([128, CT], BF16)
        w3rs_s = cpool.tile([128, CT, 512], BF16)
        tmpb_s = cpool.tile([128, CT, 2], F32)

        # h2 (GLU output) for both batch elems; col = jj*512 + n
        h2_all = [cpool.tile([128, CT, S], BF16, tag=f"h2_{b}",
                             name=f"h2_{b}") for b in range(BL)]
        def finish_combine(y0_s, y7_s, pm, tmp, emit):
            P1, M1 = pm[:, 0, :], pm[:, 1, :]
            P2, M2 = pm[:, 2, :], pm[:, 3, :]
            P3, M3 = pm[:, 4, :], pm[:, 5, :]
            t0, t1 = tmp[:, 0, :], tmp[:, 1, :]
            # j0 = Y0 + P1 + P2 + P3
            nc.vector.tensor_add(t0, y0_s, P1)
            nc.vector.tensor_add(t1, P2, P3)
            nc.vector.tensor_add(t0, t0, t1)
            emit(0, t0)
            # j2 = P1 + 4*P2 + 0.25*P3
            nc.vector.tensor_scalar(t0, P2, 4.0, None, OP.mult)
            nc.vector.tensor_add(t0, t0, P1)
            nc.vector.tensor_scalar(t1, P3, 0.25, None, OP.mult)
            nc.vector.tensor_add(t0, t0, t1)
            emit(2, t0)
            # j1 = M1 + 2*M2 + 0.5*M3
            nc.vector.tensor_scalar(t0, M2, 2.0, None, OP.mult)
            nc.vector.tensor_add(t0, t0, M1)
            nc.vector.tensor_scalar(t1, M3, 0.5, None, OP.mult)
            nc.vector.tensor_add(t0, t0, t1)
            emit(1, t0)
            # j3 = M1 + 8*M2 + 0.125*M3 + Y7
            nc.vector.tensor_scalar(t0, M2, 8.0, None, OP.mult)
            nc.vector.tensor_add(t0, t0, M1)
            nc.vector.tensor_scalar(t1, M3, 0.125, None, OP.mult)
            nc.vector.tensor_add(t1, t1, y7_s)
            nc.vector.tensor_add(t0, t0, t1)
            emit(3, t0)

        def out_combine(ps, y0_s, y7_s, yc, pm, bias_ap, t):
            """Incremental A^T combine; ps is tap t's psum [128, 256].
            DVE may read only one PSUM input, so odd taps are staged to
            SBUF (yc) by the scalar engine first."""
            if t == 0:
                nc.scalar.activation(y0_s, ps, AF.Copy)
            elif t in (1, 3, 5):
                nc.scalar.activation(yc[:, (t - 1) // 2, :], ps, AF.Copy)
            elif t in (2, 4, 6):
                k = t // 2 - 1
                nc.vector.tensor_add(pm[:, 2 * k, :], yc[:, k, :], ps)
                nc.vector.tensor_sub(pm[:, 2 * k + 1, :], yc[:, k, :], ps)
                if k == 0 and bias_ap is not None:
                    nc.vector.tensor_scalar(pm[:, 0, :], pm[:, 0, :],
                                            bias_ap, None, OP.add)
                    nc.vector.tensor_scalar(pm[:, 1, :], pm[:, 1, :],
                                            bias_ap, None, OP.add)
            elif t == 7:
                nc.scalar.activation(y7_s, ps, AF.Copy)

        HB = (range(0, 9), range(8, 17))   # blocks per half

        with tc.tile_pool(name="xtb", bufs=2) as xtpool, \
             tc.tile_pool(name="wst", bufs=3) as wpool, \
             tc.tile_pool(name="hbk", bufs=2) as hpool, \
             tc.tile_pool(name="ysb", bufs=1) as ypool, \
             tc.tile_pool(name="pmt", bufs=1) as pmpool, \
             tc.tile_pool(name="ln", bufs=1) as lnpool, \
             tc.tile_pool(name="sml", bufs=2) as smpool:

            for b in range(BL):
                sp = ExitStack()
                ypdpool = sp.enter_context(tc.tile_pool(name=f"ypd{b}", bufs=1))
                # z-padded ypad in u-layout: [c, ct, j_u, col]
                ypad = ypdpool.tile([128, CT, 4, YW], BF16, tag="ypad")
                nc.vector.memset(ypad, 0.0)

                # ============ stage 1: dcnn winograd (per half) ============
                for hf in range(2):
                    s1 = ExitStack()
                    xxpool = s1.enter_context(
                        tc.tile_pool(name=f"xxp{b}_{hf}", bufs=1))
                    xx = xxpool.tile([128, CT, 8, XW], F32R, tag="xx",
                                     name=f"xx_{b}_{hf}")
                    with nc.named_scope(f"dtf{b}_{hf}"):
                        for w in HB[hf]:
                            xtw = xtpool.tile([128, C], BF16, tag="xt")
                            nc.sync.dma_start(out=xtw, in_=xt_d[b, w])
                            lo = 31 * w - 248 * hf
                            for i in range(CT):
                                ps = pp_tf.tile([128, 8, TPB], F32, tag="tf")
                                nc.tensor.matmul(
                                    ps, xtw[:, 128 * i:128 * (i + 1)],
                                    bx_s, start=True, stop=True)
                                nc.scalar.activation(
                                    xx[:, i, :, lo:lo + 31], ps, AF.Copy)
                    with nc.named_scope(f"dmm{b}_{hf}"):
                        for jc in range(CT):
                            y0_s = ypool.tile([128, 256], F32, tag="y0")
                            y7_s = ypool.tile([128, 256], F32, tag="y7")
                            yc = ypool.tile([128, 3, 256], F32, tag="yc")
                            pm = pmpool.tile([128, 6, 256], F32, tag="pm")
                            for t in range(8):
                                wab = wpool.tile([128, CT, 128], F32R,
                                                 tag="wst")
                                nc.sync.dma_start(out=wab, in_=waf_d[jc, t])
                                ps = pp_main.tile([128, 256], F32, tag="mm")
                                for i in range(CT):
                                    nc.tensor.matmul(
                                        ps, wab[:, i, :],
                                        xx[:, i, t, 8 * hf:8 * hf + 256],
                                        start=(i == 0), stop=(i == CT - 1))
                                out_combine(ps, y0_s, y7_s, yc, pm,
                                            dcnnb_s[:, jc:jc + 1], t)
                            tmp = pmpool.tile([128, 2, 256], F32, tag="tmp")

                            def emit_y(jj, src, jc=jc, hf=hf):
                                ju = (jj + 2) % 4
                                off = (1 if jj >= 2 else 0) + 256 * hf
                                nc.scalar.activation(
                                    ypad[:, jc, ju, off:off + 256], src,
                                    AF.Copy)
                            finish_combine(y0_s, y7_s, pm, tmp, emit_y)
                    s1.close()

                # ============ stage 2: column LN ============
                with nc.named_scope(f"ln{b}"):
                    for ju in range(4):
                        off = 1 if ju < 2 else 0
                        rows = lnpool.tile([1, 4, 512], F32, tag="rows")
                        pss = pp_stat.tile([1, 512], F32, tag="cs")
                        psq = pp_stat.tile([1, 512], F32, tag="cq")
                        for i in range(CT):
                            sl = ypad[:, i, ju, off:off + 512]
                            y2 = smpool.tile([128, 512], BF16, tag="y2")
                            nc.scalar.activation(y2, sl, AF.Square)
                            nc.tensor.matmul(pss, onesc_s, sl,
                                             start=(i == 0), stop=(i == CT - 1))
                            nc.tensor.matmul(psq, onesc_s, y2,
                                             start=(i == 0), stop=(i == CT - 1))
                        mc = rows[:, 0, :]
                        nc.vector.tensor_scalar_mul(mc, pss, 1.0 / C)
                        ex2 = rows[:, 1, :]
                        nc.vector.tensor_scalar_mul(ex2, psq, 1.0 / C)
                        tmp = rows[:, 2, :]
                        nc.vector.tensor_mul(tmp, mc, mc)
                        nc.vector.tensor_sub(ex2, ex2, tmp)
                        nc.scalar.activation(tmp, ex2, AF.Ln, bias=eps1_s)
                        arow = rows[:, 3, :]
                        nc.scalar.activation(arow, tmp, AF.Exp, scale=-0.5)
                        nc.vector.tensor_mul(mc, mc, arow)
                        abrow = smpool.tile([1, 2, 512], BF16, tag="ab")
                        nc.scalar.activation(abrow[:, 0, :], arow, AF.Copy)
                        nc.scalar.activation(abrow[:, 1, :], mc, AF.Copy)
                        pa = pp_misc.tile([128, 512], F32, tag="bc")
                        pb = pp_misc.tile([128, 512], F32, tag="bc")
                        nc.tensor.matmul(pa, ones1_s, abrow[:, 0, :],
                                         start=True, stop=True)
                        nc.tensor.matmul(pb, ones1_s, abrow[:, 1, :],
                                         start=True, stop=True)
                        ab = smpool.tile([128, 512], BF16, tag="abb")
                        nc.scalar.activation(ab, pa, AF.Copy)
                        bb = smpool.tile([128, 512], BF16, tag="bbb")
                        nc.scalar.activation(bb, pb, AF.Copy)
                        for i in range(CT):
                            sl = ypad[:, i, ju, off:off + 512]
                            nc.vector.tensor_mul(sl, sl, ab)
                            nc.vector.tensor_sub(sl, sl, bb)

                # ====== stages 3-5 per half: w1+tf, conv2 main+GLU+BN ======
                for hf in range(2):
                    s2 = ExitStack()
                    xhpool = s2.enter_context(
                        tc.tile_pool(name=f"xhp{b}_{hf}", bufs=1))
                    xh = xhpool.tile([128, C2T, 8, XW], BF16, tag="xh",
                                     name=f"xh_{b}_{hf}")
                    with nc.named_scope(f"w1tf{b}_{hf}"):
                        for w in HB[hf]:
                            hblk = hpool.tile([128, 2 * C], BF16, tag="hb")
                            wcp = hpool.tile([128, CT, 4, 32], BF16,
                                             tag="wcp")
                            for i in range(CT):
                                nc.scalar.activation(
                                    wcp[:, i],
                                    ypad[:, i, :, 31 * w:31 * w + 32],
                                    AF.Copy)
                            for coh in range(2):
                                psw = pp_main.tile([128, 512], F32, tag="mm")
                                for i in range(CT):
                                    nc.tensor.matmul(
                                        psw, wcp[:, i],
                                        w1t_s[:, i, 512 * coh:512 * (coh + 1)],
                                        start=(i == 0), stop=(i == CT - 1))
                                hpre = smpool.tile([128, 512], BF16, tag="hp")
                                nc.vector.tensor_add(
                                    hpre, psw,
                                    b1bc_s[:, 512 * coh:512 * (coh + 1)])
                                nc.scalar.activation(
                                    hblk[:, 512 * coh:512 * (coh + 1)], hpre,
                                    AF.Silu)
                            lo = 31 * w - 248 * hf
                            for i2 in range(C2T):
                                ps = pp_tf.tile([128, 8, TPB], F32, tag="tf")
                                nc.tensor.matmul(
                                    ps, hblk[:, 128 * i2:128 * (i2 + 1)],
                                    bh_s, start=True, stop=True)
                                nc.scalar.activation(
                                    xh[:, i2, :, lo:lo + 31], ps, AF.Copy)

                    with nc.named_scope(f"c2mm{b}_{hf}"):
                        for j2 in range(C2T):
                            y0_s = ypool.tile([128, 256], F32, tag="y0")
                            y7_s = ypool.tile([128, 256], F32, tag="y7")
                            yc = ypool.tile([128, 3, 256], F32, tag="yc")
                            pm = pmpool.tile([128, 6, 256], F32, tag="pm")
                            is_a = j2 < CT
                            bias = b2_s[:, j2:j2 + 1] if is_a else None
                            for t in range(8):
                                w2b = wpool.tile([128, C2T, 128], BF16,
                                                 tag="wst")
                                nc.sync.dma_start(out=w2b, in_=w2f_d[j2, t])
                                ps = pp_main.tile([128, 256], F32, tag="mm")
                                for i2 in range(C2T):
                                    nc.tensor.matmul(
                                        ps, w2b[:, i2, :],
                                        xh[:, i2, t, 8 * hf:8 * hf + 256],
                                        start=(i2 == 0),
                                        stop=(i2 == C2T - 1))
                                out_combine(ps, y0_s, y7_s, yc, pm, bias, t)
                            tmp = pmpool.tile([128, 2, 256], F32, tag="tmp")
                            if is_a:
                                def emit_a(jj, src, j2=j2, hf=hf):
                                    nc.scalar.activation(
                                        h2_all[b][:, j2,
                                                  512 * jj + 256 * hf:
                                                  512 * jj + 256 * hf + 256],
                                        src, AF.Copy)
                                finish_combine(y0_s, y7_s, pm, tmp, emit_a)
                            else:
                                jg = j2 - CT

                                def emit_g(jj, src, jg=jg, hf=hf):
                                    sg = smpool.tile([128, 256], BF16,
                                                     tag="sg")
                                    nc.scalar.activation(
                                        sg, src, AF.Sigmoid,
                                        bias=b2_s[:, CT + jg:CT + jg + 1])
                                    h2sl = h2_all[b][:, jg,
                                                     512 * jj + 256 * hf:
                                                     512 * jj + 256 * hf + 256]
                                    nc.vector.tensor_mul(h2sl, h2sl, sg)
                                    nc.vector.bn_stats(
                                        st6[jg][:, b * 8 + hf * 4 + jj, :],
                                        h2sl)
                                finish_combine(y0_s, y7_s, pm, tmp, emit_g)
                                if b == BL - 1 and hf == 1:
                                    mv = smpool.tile([128, 512], F32,
                                                     tag="mv", name=f"mv{jg}")
                                    nc.vector.bn_aggr(mv[:, 0:2], st6[jg])
                                    nc.vector.tensor_mul(mv[:, 2:3],
                                                         mv[:, 0:1],
                                                         mv[:, 0:1])
                                    nc.vector.tensor_add(mv[:, 3:4],
                                                         mv[:, 1:2],
                                                         mv[:, 2:3])
                                    nc.vector.tensor_scalar_mul(
                                        bnpack_s[:, jg, 0:1], mv[:, 0:1],
                                        float(BL * S))
                                    nc.vector.tensor_scalar_mul(
                                        bnpack_s[:, jg, 1:2], mv[:, 3:4],
                                        float(BL * S))
                                    nc.sync.dma_start(out=bn_in[jg].ap(),
                                                      in_=bnpack_s[:, jg, :])
                                    cc = nc.gpsimd.collective_compute(
                                        "AllReduce", OP.add,
                                        replica_groups=[list(range(NCORES))],
                                        ins=[bn_in[jg].ap()],
                                        outs=[bn_out[jg].ap()])
                                    rd = nc.sync.dma_start(
                                        out=bnsum_s[:, jg, :],
                                        in_=bn_out[jg].ap())
                                    tile.add_dep_helper(rd.ins, cc.ins,
                                                        sync=True,
                                                        reason="bn ar->read")
                                    nc.vector.tensor_scalar_mul(
                                        mu_s[:, jg:jg + 1],
                                        bnsum_s[:, jg, 0:1], 1.0 / (B * S))
                                    nc.vector.tensor_scalar_mul(
                                        rs_s[:, jg:jg + 1],
                                        bnsum_s[:, jg, 1:2], 1.0 / (B * S))
                                    nc.vector.tensor_mul(tmpb_s[:, jg, 0:1],
                                                         mu_s[:, jg:jg + 1],
                                                         mu_s[:, jg:jg + 1])
                                    nc.vector.tensor_sub(rs_s[:, jg:jg + 1],
                                                         rs_s[:, jg:jg + 1],
                                                         tmpb_s[:, jg, 0:1])
                                    nc.scalar.activation(tmpb_s[:, jg, 1:2],
                                                         rs_s[:, jg:jg + 1],
                                                         AF.Ln, bias=epsb_s)
                                    nc.scalar.activation(rs_s[:, jg:jg + 1],
                                                         tmpb_s[:, jg, 1:2],
                                                         AF.Exp, scale=-0.5)
                                    nc.scalar.activation(
                                        w3rs_s[:, jg, :], w3t_s[:, jg, :],
                                        AF.Copy, scale=rs_s[:, jg:jg + 1])
                                    nc.vector.tensor_mul(tmpb_s[:, jg, 0:1],
                                                         mu_s[:, jg:jg + 1],
                                                         rs_s[:, jg:jg + 1])
                                    nc.scalar.activation(
                                        murs_s[:, jg:jg + 1],
                                        tmpb_s[:, jg, 0:1], AF.Copy)
                    s2.close()
                sp.close()

        # ========== stage D: w3 with BN folded ==========
        with tc.tile_pool(name="ost", bufs=3) as opool, \
             tc.tile_pool(name="qrow", bufs=1) as qpool:
            with nc.named_scope("qprep"):
                psq = pp_stat.tile([1, 512], F32, tag="cs")
                for i in range(CT):
                    nc.tensor.matmul(psq, murs_s[:, i:i + 1], w3t_s[:, i, :],
                                     start=(i == 0), stop=(i == CT - 1))
                qrow = qpool.tile([1, 2, 512], F32)
                nc.vector.tensor_sub(qrow[:, 0, :], b3_s, psq)
                qbrow = qpool.tile([1, 512], BF16)
                nc.scalar.activation(qbrow, qrow[:, 0, :], AF.Copy)
                psb = pp_misc.tile([128, 512], F32, tag="bc")
                nc.tensor.matmul(psb, ones1_s, qbrow, start=True, stop=True)
                qb = qpool.tile([128, 512], F32)
                nc.scalar.activation(qb, psb, AF.Copy)

            with nc.named_scope("w3"):
                for b in range(BL):
                    for sb in range(S // 128):
                        ps = pp_main.tile([128, 512], F32, tag="mm")
                        for i in range(CT):
                            nc.tensor.matmul(
                                ps, h2_all[b][:, i, sb * 128:(sb + 1) * 128],
                                w3rs_s[:, i, :],
                                start=(i == 0), stop=(i == CT - 1))
                        ot = opool.tile([128, 512], F32, tag="o")
                        nc.vector.tensor_add(ot, ps, qb)
                        nc.sync.dma_start(
                            out=out_d[b, sb // 4,
                                      (sb % 4) * 128:(sb % 4 + 1) * 128, :],
                            in_=ot)

    nc.compile()
    return nc


def _round_f22(a):
    ai = np.ascontiguousarray(a, np.float32).view(np.uint32)
    return ((ai + (1 << 9)) & ~np.uint32((1 << 10) - 1)).view(np.float32)


def _prep(inputs):
    import ml_dtypes
    bf16 = ml_dtypes.bfloat16

    x = np.asarray(inputs["x"], np.float32)
    dcnn_w = np.asarray(inputs["dcnn_w"], np.float32)
    dcnn_b = np.asarray(inputs["dcnn_b"], np.float32)
    ln_g = np.asarray(inputs["ln_g"], np.float32)
    ln_b = np.asarray(inputs["ln_b"], np.float32)
    w1 = np.asarray(inputs["w1"], np.float32)[:, :, 0]
    b1 = np.asarray(inputs["b1"], np.float32)
    w2 = np.asarray(inputs["w2"], np.float32)
    b2 = np.asarray(inputs["b2"], np.float32)
    bn_g = np.asarray(inputs["bn_g"], np.float32)
    bn_b = np.asarray(inputs["bn_b"], np.float32)
    w3 = np.asarray(inputs["w3"], np.float32)[:, :, 0]
    b3 = np.asarray(inputs["b3"], np.float32)

    # xt[b, w, p, c] = x[b, u-2, c], u = 124w + p (0 outside [2, 2050))
    xt = np.zeros((B, NB, 128, C), np.float32)
    u = (124 * np.arange(NB))[:, None] + np.arange(128)[None, :]   # (NB,128)
    s_idx = u - 2
    valid = (s_idx >= 0) & (s_idx < S)
    sv = np.clip(s_idx, 0, S - 1)
    xt[:, valid] = x[:, sv[valid], :]
    xt = xt.astype(bf16)

    # dcnn winograd weights: waf[jc, t, p, i, co'] =
    #   sum_k G[t,k] dcnn_w[128*jc+co', 128*i+p, k]
    wag = np.einsum("tk,oik->toi", G_W, dcnn_w)            # (8, C, C)
    waf = wag.reshape(8, CT, 128, CT, 128)                  # t, jc, co', i, p
    waf = np.ascontiguousarray(waf.transpose(1, 0, 4, 3, 2))  # jc,t,p,i,co'
    waf = _round_f22(waf)

    # w1 (ln folded), transposed: w1t[p, i, o] = w1f[o, 128i+p]
    w1f = w1 * ln_g[None, :]
    b1p = b1 + w1 @ ln_b
    w1tt = np.ascontiguousarray(
        w1f.reshape(2 * C, CT, 128).transpose(2, 1, 0)).astype(bf16)
    b1bc = np.broadcast_to(b1p[None, :], (128, 2 * C)).astype(np.float32)
    b1bc = np.ascontiguousarray(b1bc)

    # conv2 winograd weights: w2f[j2, t, p, i2, co'] =
    #   sum_k G[t,k] w2[128*j2+co', 128*i2+p, k]
    w2g = np.einsum("tk,oik->toi", G_W, w2)                 # (8, 2C, 2C)
    w2f = w2g.reshape(8, C2T, 128, C2T, 128)                # t, j2, co', i2, p
    w2f = np.ascontiguousarray(w2f.transpose(1, 0, 4, 3, 2)).astype(bf16)

    # w3 (bn affine folded)
    w3f = w3 * bn_g[None, :]
    b3p = b3 + w3 @ bn_b
    w3t = np.ascontiguousarray(
        w3f.reshape(512, CT, 128).transpose(2, 1, 0)).astype(bf16)

    # B^T banded constants
    BTb = BT_W.astype(np.float32)
    bxm = np.zeros((128, 8, TPB), np.float32)
    for p in range(128):
        for nn in range(TPB):
            d = p - 4 * nn
            if 0 <= d < 8:
                bxm[p, :, nn] = BTb[:, d]
    bhm = np.zeros((128, 8, TPB), np.float32)
    for ju in range(4):
        for qq in range(32):
            p = 32 * ju + qq
            uo = 4 * qq + ju
            for nn in range(TPB):
                d = uo - 4 * nn
                if 0 <= d < 8:
                    bhm[p, :, nn] = BTb[:, d]
    bxm = bxm.reshape(128, NTF).astype(bf16)
    bhm = bhm.reshape(128, NTF).astype(bf16)

    common = {
        "waf": waf,
        "w1t": w1tt,
        "w2f": w2f,
        "w3t": w3t,
        "bx": np.ascontiguousarray(bxm),
        "bh": np.ascontiguousarray(bhm),
        "b1bc": b1bc,
        "dcnnb": np.ascontiguousarray(dcnn_b.reshape(CT, 128).T),
        "b2p": np.ascontiguousarray(b2.reshape(C2T, 128).T),
        "b3p": np.ascontiguousarray(b3p.reshape(1, 512)),
    }
    in_maps = []
    for c in range(NCORES):
        m = dict(common)
        m["xt"] = np.ascontiguousarray(xt[c * BL:(c + 1) * BL])
        in_maps.append(m)
    return in_maps


def kernel(**inputs) -> np.ndarray:
    global LAST_RESULT, _NC
    if _NC is None:
        _NC = _build()
    in_maps = _prep(inputs)
    res = run_bass_kernel_spmd(_NC, in_maps, list(range(NCORES)))
    LAST_RESULT = res
    out = np.empty((B, S, C), np.float32)
    sperm = (4 * np.arange(NT)[None, :] + np.arange(4)[:, None]).reshape(-1)
    for c in range(NCORES):
        o = np.asarray(res.results[c]["out"]).reshape(BL, 4 * NT, C)
        out[c * BL:(c + 1) * BL, sperm, :] = o
    return out
